# revision 10
# baseline (speedup 1.0000x reference)
"""Swin-style window attention kernel for Trainium2 (8 NeuronCores, data-parallel).

Computes, for x:[2048,49,384]:
    qkv = x @ qkv_w.T + qkv_b ; split into q,k,v heads (12 x 32)
    attn = softmax(q k^T / sqrt(32) + rel_pos_bias + window_mask)
    out  = (attn @ v) @ proj_w.T

Strategy: data-parallel over the leading B_ axis (256 windows / core).
On-chip layout is channel-major (x pre-transposed on host), windows are
processed in pairs (98 tokens) so attention matmuls use 98-wide tiles.
Relative-position bias + window mask are folded into one multiplicative
term EB = exp(bias + mask) precomputed on the host; softmax is computed
without max-subtraction (scores are O(1) here) as exp(s)*EB / colsum.
All matmuls run in bf16 with fp32 PSUM accumulation.

Software-pipelined schedule: the per-pair attention chain
(scores[PE] -> exp[ACT] -> *EB[DVE] -> colsum/AV[PE] -> 1/Z[DVE] ->
normalize[DVE]) is interleaved, with a 2-slot lag between scores and
colsum/AV, with the projection GEMMs of the next block and the output
projection of the previous block, so every engine's in-order queue
always has independent work.  PSUM->SBUF evacuation copies are
round-robined between the Scalar and Vector engines to balance load.
"""

import itertools
import sys

sys.path.insert(0, "/opt/trn_rl_repo")

import numpy as np
import ml_dtypes

import concourse.bacc as bacc
import concourse.mybir as mybir
import concourse.tile as tile
from concourse.bass_utils import run_bass_kernel_spmd

BF16 = ml_dtypes.bfloat16
F32 = np.float32

N_CORES = 8
D, H, HD = 384, 12, 32
WN = 49                      # tokens per window
NW = 64                      # distinct window masks
B_ = 2048
B_CORE = B_ // N_CORES       # 256 windows per core
T_CORE = B_CORE * WN         # 12544 tokens per core
PT = 2 * WN                  # 98 tokens per window pair
N_PAIR = B_CORE // 2         # 128 pairs per core
PAIR_PAT = NW // 2           # 32 distinct pair mask patterns
BLK_PAIRS = 8
BLK_T = BLK_PAIRS * PT       # 784 tokens per block
N_BLK = N_PAIR // BLK_PAIRS  # 16 blocks per core
NH = BLK_T // 2              # 392: half-block free dim for 512-limit psum
SCALE = HD ** (-0.5)
LAG = 2                      # slots between scores and colsum/AV of a pair

_BF = mybir.dt.bfloat16
_F32 = mybir.dt.float32


def _relative_position_index():
    coords = np.stack(np.meshgrid(np.arange(7), np.arange(7), indexing="ij"))
    cf = coords.reshape(2, -1)
    rel = cf[:, :, None] - cf[:, None, :]
    rel = rel.transpose(1, 2, 0).copy()
    rel[:, :, 0] += 6
    rel[:, :, 1] += 6
    rel[:, :, 0] *= 13
    return rel.sum(-1)  # [49, 49] int


def _build_nc(qkv_bias_nonzero: bool):
    nc = bacc.Bacc("TRN2", target_bir_lowering=False, debug=True)

    xT_d = nc.dram_tensor("xT", [D, T_CORE], _BF, kind="ExternalInput")
    wqk_d = nc.dram_tensor("wqk", [128, 3, 2 * D], _BF, kind="ExternalInput")
    wv_d = nc.dram_tensor("wv", [128, 3, D], _BF, kind="ExternalInput")
    pw_d = nc.dram_tensor("pw", [128, 3, D], _BF, kind="ExternalInput")
    eb_d = nc.dram_tensor("eb", [PT, PAIR_PAT, H, PT], _BF, kind="ExternalInput")
    bqk_d = nc.dram_tensor("bqk", [1, 2 * D], _BF, kind="ExternalInput")
    bv_d = nc.dram_tensor("bv", [1, D], _BF, kind="ExternalInput")
    yT_d = nc.dram_tensor("yT", [D, T_CORE], _BF, kind="ExternalOutput")

    xT_view = xT_d[:, :].rearrange("(k p) t -> p k t", p=128)
    yT_view = yT_d[:, :].rearrange("(k p) t -> p k t", p=128)

    with tile.TileContext(nc) as tc:
        with (
            tc.tile_pool(name="consts", bufs=1) as consts,
            tc.tile_pool(name="xin", bufs=3) as xin,
            tc.tile_pool(name="qkp", bufs=2) as qkp,
            tc.tile_pool(name="vp", bufs=2) as vp,
            tc.tile_pool(name="attnp", bufs=4) as attnp,
            tc.tile_pool(name="up", bufs=2) as up,
            tc.tile_pool(name="outp", bufs=2) as outp,
            tc.tile_pool(name="yp", bufs=2) as yp,
            # PSUM: s 4 banks + cs 1 + o 1 + mm 2 = 8 banks exactly
            tc.tile_pool(name="ps_s", bufs=1, space="PSUM") as ps_s,
            tc.tile_pool(name="ps_cs", bufs=1, space="PSUM") as ps_cs,
            tc.tile_pool(name="ps_o", bufs=1, space="PSUM") as ps_o,
            tc.tile_pool(name="ps_mm", bufs=2, space="PSUM") as ps_mm,
        ):
            # ---- constants ----
            # order matters: the sync DMA queue serializes, so load what the
            # prologue projections need (weights + xT(0)) before the big eb
            # table.  eb itself is chunked so block 0's patterns arrive early.
            wqk_sb = consts.tile([128, 3, 2 * D], _BF)
            nc.sync.dma_start(out=wqk_sb, in_=wqk_d[:, :, :])
            wv_sb = consts.tile([128, 3, D], _BF)
            nc.sync.dma_start(out=wv_sb, in_=wv_d[:, :, :])
            pw_sb = consts.tile([128, 3, D], _BF)
            nc.sync.dma_start(out=pw_sb, in_=pw_d[:, :, :])
            ones_sb = consts.tile([PT, 32], _BF)
            nc.vector.memset(ones_sb, 1.0)
            eb_sb = consts.tile([PT, PAIR_PAT, H, PT], _BF)
            if qkv_bias_nonzero:
                bqk_sb = consts.tile([1, 2 * D], _BF)
                nc.sync.dma_start(out=bqk_sb, in_=bqk_d[:, :])
                bv_sb = consts.tile([1, D], _BF)
                nc.sync.dma_start(out=bv_sb, in_=bv_d[:, :])
                onetok_sb = consts.tile([1, NH], _BF)
                nc.vector.memset(onetok_sb, 1.0)

            # PSUM->SBUF copy dispatcher: 8/13 scalar, 5/13 vector
            copy_cycle = itertools.cycle(
                [0, 1, 0, 0, 1, 0, 1, 0, 0, 1, 0, 1, 0]
            )

            def evac(out, in_):
                if next(copy_cycle) == 0:
                    nc.scalar.copy(out=out, in_=in_)
                else:
                    nc.vector.tensor_copy(out=out, in_=in_)

            # ---- per-block tile state ----
            xT_sb = [None] * N_BLK
            qk_sb = [None] * N_BLK
            v_sb = [None] * N_BLK
            outN_sb = [None] * N_BLK
            yT_sb = [None] * N_BLK
            attn_of = {}   # pair index -> attn tile
            ebmul_pending = {}   # pair index -> (attn_out, attn_exp)

            def dma_x(b):
                xT_sb[b] = xin.tile([128, 3, BLK_T], _BF, tag="x", name=f"xT{b}")
                # one DMA per k-chunk: transfers spread across queues and
                # consumers (which read a single k slice) unblock earlier
                for k in range(3):
                    nc.sync.dma_start(
                        out=xT_sb[b][:, k, :],
                        in_=xT_view[:, k, b * BLK_T : (b + 1) * BLK_T],
                    )

            def qk_group(b, m, nh):
                mm_ps = ps_mm.tile([128, NH], _F32, tag="mm", name=f"mm{b}_{m}_{nh}")
                for k in range(3):
                    nc.tensor.matmul(
                        out=mm_ps,
                        lhsT=wqk_sb[:, k, 128 * m : 128 * (m + 1)],
                        rhs=xT_sb[b][:, k, nh * NH : (nh + 1) * NH],
                        start=(k == 0),
                        stop=(k == 2) if not qkv_bias_nonzero else False,
                    )
                if qkv_bias_nonzero:
                    nc.tensor.matmul(
                        out=mm_ps,
                        lhsT=bqk_sb[:, 128 * m : 128 * (m + 1)],
                        rhs=onetok_sb,
                        start=False,
                        stop=True,
                    )
                evac(qk_sb[b][:, m, nh * NH : (nh + 1) * NH], mm_ps)

            def v_group(b, p8):
                v_ps = ps_mm.tile([PT, D], _F32, tag="mm", name="v_ps")
                for k in range(3):
                    nc.tensor.matmul(
                        out=v_ps,
                        lhsT=xT_sb[b][:, k, p8 * PT : (p8 + 1) * PT],
                        rhs=wv_sb[:, k, :],
                        start=(k == 0),
                        stop=(k == 2) if not qkv_bias_nonzero else False,
                    )
                if qkv_bias_nonzero:
                    nc.tensor.matmul(
                        out=v_ps,
                        lhsT=onetok_sb[:, :PT],
                        rhs=bv_sb,
                        start=False,
                        stop=True,
                    )
                evac(v_sb[b][:, p8, :], v_ps)

            def proj_group(b, m, nh):
                y_ps = ps_mm.tile([128, NH], _F32, tag="mm", name="y_ps")
                for k in range(3):
                    nc.tensor.matmul(
                        out=y_ps,
                        lhsT=pw_sb[:, k, 128 * m : 128 * (m + 1)],
                        rhs=outN_sb[b][:, k, nh * 4 : (nh + 1) * 4, :],
                        start=(k == 0),
                        stop=(k == 2),
                    )
                evac(yT_sb[b][:, m, nh * NH : (nh + 1) * NH], y_ps)

            def scores_and_exp(i):
                """PE scores + ACT exp + DVE *EB for pair i."""
                b, p8 = i // 8, i % 8
                ts = p8 * PT
                # one 4-bank PSUM tile; bank j <- row-group j only
                # (3 used regions per bank at 512B offsets), so concurrent
                # row-groups hit different banks.
                s_ps = ps_s.tile([PT, 4, 4, 128], _F32, tag="s", name="s4")
                for r in range(3):
                    for j in range(4):
                        g = (j + r) % 3
                        nc.tensor.matmul(
                            out=s_ps[:, j, g, :PT],
                            lhsT=qk_sb[b][
                                32 * j : 32 * (j + 1), 3 + g, ts : ts + PT
                            ],
                            rhs=qk_sb[b][32 * j : 32 * (j + 1), g, ts : ts + PT],
                            start=True,
                            stop=True,
                            tile_position=(32 * j, 0),
                        )
                # one merged exp over all 12 heads (attn idx 3j+g = head 4g+j)
                attn_e = attnp.tile([PT, H, PT], _BF, tag="attne", name=f"attne{i}")
                attn_v = attn_e.rearrange("p (j g) c -> p j g c", g=3)
                nc.scalar.activation(
                    out=attn_v,
                    in_=s_ps[:, :, :3, :PT],
                    func=mybir.ActivationFunctionType.Exp,
                )
                # out-of-place *EB keeps the DVE in its fast 2x mode; the
                # multiply itself is emitted at the END of the slot (see the
                # main loop) so the DVE's in-order queue never stalls on
                # exp(i) while ready work (recip/norm of pair i-LAG) waits.
                attn_sb = attnp.tile([PT, H, PT], _BF, tag="attn", name=f"attn{i}")
                attn_of[i] = attn_sb
                ebmul_pending[i] = (attn_sb, attn_e)

            def reduce_pair(j):
                """colsum/AV on PE + 1/Z + normalize on DVE for pair j."""
                b, p8 = j // 8, j % 8
                attn_sb = attn_of.pop(j)
                # softmax denominators: 4 col-tiled matmuls, 3 heads each
                cs_ps = ps_cs.tile([128, 3, PT], _F32, tag="cs", name=f"cs{j}")
                for c in range(4):
                    nc.tensor.matmul(
                        out=cs_ps[32 * c : 32 * (c + 1), :, :],
                        lhsT=ones_sb,
                        rhs=attn_sb[:, 3 * c : 3 * (c + 1), :],
                        start=True,
                        stop=True,
                        tile_position=(0, 32 * c),
                    )
                u_sb = up.tile([128, 3, PT], _F32, tag="u", name=f"u{j}")
                nc.vector.reciprocal_approx_fast(out=u_sb, in_=cs_ps)

                # attn @ v (unnormalized), col-tiled by head
                o_ps = ps_o.tile([128, 3, PT], _F32, tag="o", name=f"o{j}")
                for r in range(4):
                    for g in range(3):
                        jj = (g + r) % 4
                        h = 4 * g + jj
                        nc.tensor.matmul(
                            out=o_ps[32 * jj : 32 * (jj + 1), g, :],
                            lhsT=v_sb[b][:, p8, 32 * h : 32 * (h + 1)],
                            rhs=attn_sb[:, 3 * jj + g, :],
                            start=True,
                            stop=True,
                            tile_position=(0, 32 * jj),
                        )
                # normalize: outN = o * (1/Z)
                nc.vector.tensor_mul(outN_sb[b][:, :, p8, :], o_ps, u_sb)

            def alloc_block(b):
                qk_sb[b] = qkp.tile([128, 6, BLK_T], _BF, tag="qk", name=f"qk{b}")
                v_sb[b] = vp.tile([PT, BLK_PAIRS, D], _BF, tag="v", name=f"v{b}")

            def alloc_out(b):
                outN_sb[b] = outp.tile([128, 3, BLK_PAIRS, PT], _BF, tag="outN", name=f"outN{b}")
                yT_sb[b] = yp.tile([128, 3, BLK_T], _BF, tag="yT", name=f"yT{b}")

            # ---- prologue: block 0 projections, first DMAs ----
            dma_x(0)
            nc.sync.dma_start(out=eb_sb[:, 0:8], in_=eb_d[:, 0:8])
            if N_BLK > 1:
                dma_x(1)
            for c in range(1, 4):
                nc.sync.dma_start(
                    out=eb_sb[:, 8 * c : 8 * (c + 1)],
                    in_=eb_d[:, 8 * c : 8 * (c + 1)],
                )
            alloc_block(0)
            for m in range(6):
                qk_group(0, m, 0)
            for p8 in range(4):
                v_group(0, p8)
            for m in range(6):
                qk_group(0, m, 1)
            for p8 in range(4, 8):
                v_group(0, p8)

            # ---- pipelined main loop ----
            for B in range(N_BLK + 1):
                if B + 2 < N_BLK:
                    dma_x(B + 2)
                if B + 1 < N_BLK:
                    alloc_block(B + 1)
                if B < N_BLK:
                    alloc_out(B)

                # interleavable work units for this iteration
                units = []
                if B + 1 < N_BLK:
                    for m in range(6):
                        units.append(("qk", B + 1, m, 0))
                if B >= 1:
                    for m in range(3):
                        units.append(("proj", B - 1, m, 0))
                if B + 1 < N_BLK:
                    for p8 in range(4):
                        units.append(("v", B + 1, p8, None))
                    for m in range(6):
                        units.append(("qk", B + 1, m, 1))
                if B >= 1:
                    for m in range(3):
                        units.append(("proj", B - 1, m, 1))
                if B + 1 < N_BLK:
                    for p8 in range(4, 8):
                        units.append(("v", B + 1, p8, None))

                nu = len(units)
                for s in range(8):
                    i = 8 * B + s
                    if i < N_PAIR:
                        scores_and_exp(i)
                    j = i - LAG
                    if 0 <= j < N_PAIR:
                        reduce_pair(j)
                    for u in units[(nu * s + 7) // 8 : (nu * (s + 1) + 7) // 8]:
                        kind, bb, m, nh = u
                        if kind == "qk":
                            qk_group(bb, m, nh)
                        elif kind == "v":
                            v_group(bb, m)
                        else:
                            proj_group(bb, m, nh)
                    if i in ebmul_pending:
                        attn_sb, attn_e = ebmul_pending.pop(i)
                        pr = i % PAIR_PAT
                        nc.vector.tensor_mul(attn_sb, attn_e, eb_sb[:, pr, :, :])

                if B >= 1:
                    nc.sync.dma_start(
                        out=yT_view[:, :, (B - 1) * BLK_T : B * BLK_T],
                        in_=yT_sb[B - 1],
                    )

    nc.compile()
    return nc


_NC_CACHE: dict = {}


def _get_nc(qkv_bias_nonzero: bool):
    key = qkv_bias_nonzero
    if key not in _NC_CACHE:
        _NC_CACHE[key] = _build_nc(qkv_bias_nonzero)
    return _NC_CACHE[key]


def _host_prep(x, mask, qkv_w, qkv_b, proj_w, rpb_table):
    """Build per-core input maps (all device tensors bf16)."""
    # x^T per core: [384, 12544], channel-major
    x8 = np.ascontiguousarray(x, dtype=F32).reshape(N_CORES, T_CORE, D)

    # weights: lhsT layout [ci, co] chunked as [128, 3, co]
    wqkv_t = np.ascontiguousarray(qkv_w, dtype=F32).T  # [384, 1152] = [ci, co]
    wqk = wqkv_t[:, : 2 * D].copy()
    wqk[:, :D] *= SCALE  # fold 1/sqrt(hd) into q weights
    wv = wqkv_t[:, 2 * D :]
    pw_t = np.ascontiguousarray(proj_w, dtype=F32).T  # [ci, co]

    def chunk(w):  # [384, co] -> [128, 3, co]
        return np.ascontiguousarray(
            w.reshape(3, 128, w.shape[1]).transpose(1, 0, 2)
        ).astype(BF16)

    wqk_a, wv_a, pw_a = chunk(wqk), chunk(wv), chunk(pw_t)

    # EB = exp(biasT + maskT) per pair pattern, [98, 32, 12, 98] (j, pr, h, i)
    rpi = _relative_position_index()
    bias = np.asarray(rpb_table, dtype=F32)[rpi]          # [i, j, H]
    biasT = bias.transpose(2, 1, 0)                        # [H, j, i]
    maskT = np.asarray(mask, dtype=F32).transpose(0, 2, 1)  # [w, j, i]
    mb = np.full((PAIR_PAT, H, PT, PT), -30000.0, dtype=F32)
    mb[:, :, :WN, :WN] = biasT[None] + maskT[0::2, None, :, :]
    mb[:, :, WN:, WN:] = biasT[None] + maskT[1::2, None, :, :]
    eb = np.exp(mb)
    # device head order is (j, g): idx = 3j + g holds head h = 4g + j
    perm = np.array([4 * (i % 3) + i // 3 for i in range(H)])
    eb = eb[:, perm]
    eb_a = np.ascontiguousarray(eb.transpose(2, 0, 1, 3)).astype(BF16)

    b = np.asarray(qkv_b, dtype=F32)
    bqk = b[: 2 * D].copy()
    bqk[:D] *= SCALE
    bqk_a = bqk[None, :].astype(BF16)
    bv_a = b[2 * D :][None, :].astype(BF16)

    in_maps = []
    for c in range(N_CORES):
        xT_c = np.ascontiguousarray(x8[c].T).astype(BF16)  # [384, 12544]
        in_maps.append(
            {
                "xT": xT_c,
                "wqk": wqk_a,
                "wv": wv_a,
                "pw": pw_a,
                "eb": eb_a,
                "bqk": bqk_a,
                "bv": bv_a,
            }
        )
    return in_maps


def kernel(x, mask, qkv_w, qkv_b, proj_w, rpb_table, _want_trace=False):
    qkv_bias_nonzero = bool(np.any(np.asarray(qkv_b) != 0))
    nc = _get_nc(qkv_bias_nonzero)
    in_maps = _host_prep(x, mask, qkv_w, qkv_b, proj_w, rpb_table)
    res = run_bass_kernel_spmd(
        nc, in_maps, core_ids=list(range(N_CORES)), trace=_want_trace
    )
    yT = np.stack([res.results[c]["yT"].astype(F32) for c in range(N_CORES)])
    y = yT.transpose(0, 2, 1).reshape(B_, WN, D)
    if _want_trace:
        kernel._last_result = res
    return y


# revision 11
# speedup vs baseline: 1.0139x; 1.0139x over previous
"""Swin-style window attention kernel for Trainium2 (8 NeuronCores, data-parallel).

Computes, for x:[2048,49,384]:
    qkv = x @ qkv_w.T + qkv_b ; split into q,k,v heads (12 x 32)
    attn = softmax(q k^T / sqrt(32) + rel_pos_bias + window_mask)
    out  = (attn @ v) @ proj_w.T

Strategy: data-parallel over the leading B_ axis (256 windows / core).
On-chip layout is channel-major (x pre-transposed on host), windows are
processed in pairs (98 tokens) so attention matmuls use 98-wide tiles.
Relative-position bias + window mask are folded into one multiplicative
term EB = exp(bias + mask) precomputed on the host; softmax is computed
without max-subtraction (scores are O(1) here) as exp(s)*EB / colsum.
All matmuls run in bf16 with fp32 PSUM accumulation.

Software-pipelined schedule: the per-pair attention chain
(scores[PE] -> exp[ACT] -> *EB[DVE] -> colsum/AV[PE] -> 1/Z[DVE] ->
normalize[DVE]) is interleaved, with a 2-slot lag between scores and
colsum/AV, with the projection GEMMs of the next block and the output
projection of the previous block, so every engine's in-order queue
always has independent work.  PSUM->SBUF evacuation copies are
round-robined between the Scalar and Vector engines to balance load.
"""

import itertools
import sys

sys.path.insert(0, "/opt/trn_rl_repo")

import numpy as np
import ml_dtypes

import concourse.bacc as bacc
import concourse.mybir as mybir
import concourse.tile as tile
from concourse.bass_utils import run_bass_kernel_spmd

BF16 = ml_dtypes.bfloat16
F32 = np.float32

N_CORES = 8
D, H, HD = 384, 12, 32
WN = 49                      # tokens per window
NW = 64                      # distinct window masks
B_ = 2048
B_CORE = B_ // N_CORES       # 256 windows per core
T_CORE = B_CORE * WN         # 12544 tokens per core
PT = 2 * WN                  # 98 tokens per window pair
N_PAIR = B_CORE // 2         # 128 pairs per core
PAIR_PAT = NW // 2           # 32 distinct pair mask patterns
BLK_PAIRS = 8
BLK_T = BLK_PAIRS * PT       # 784 tokens per block
N_BLK = N_PAIR // BLK_PAIRS  # 16 blocks per core
NH = BLK_T // 2              # 392: half-block free dim for 512-limit psum
SCALE = HD ** (-0.5)
LAG = 2                      # slots between scores and colsum/AV of a pair

_BF = mybir.dt.bfloat16
_F32 = mybir.dt.float32


def _relative_position_index():
    coords = np.stack(np.meshgrid(np.arange(7), np.arange(7), indexing="ij"))
    cf = coords.reshape(2, -1)
    rel = cf[:, :, None] - cf[:, None, :]
    rel = rel.transpose(1, 2, 0).copy()
    rel[:, :, 0] += 6
    rel[:, :, 1] += 6
    rel[:, :, 0] *= 13
    return rel.sum(-1)  # [49, 49] int


def _build_nc(qkv_bias_nonzero: bool):
    nc = bacc.Bacc("TRN2", target_bir_lowering=False, debug=True)

    xT_d = nc.dram_tensor("xT", [D, T_CORE], _BF, kind="ExternalInput")
    wqk_d = nc.dram_tensor("wqk", [128, 3, 2 * D], _BF, kind="ExternalInput")
    wv_d = nc.dram_tensor("wv", [128, 3, D], _BF, kind="ExternalInput")
    pw_d = nc.dram_tensor("pw", [128, 3, D], _BF, kind="ExternalInput")
    eb_d = nc.dram_tensor("eb", [PT, PAIR_PAT, H, PT], _BF, kind="ExternalInput")
    bqk_d = nc.dram_tensor("bqk", [1, 2 * D], _BF, kind="ExternalInput")
    bv_d = nc.dram_tensor("bv", [1, D], _BF, kind="ExternalInput")
    yT_d = nc.dram_tensor("yT", [D, T_CORE], _BF, kind="ExternalOutput")

    xT_view = xT_d[:, :].rearrange("(k p) t -> p k t", p=128)
    yT_view = yT_d[:, :].rearrange("(k p) t -> p k t", p=128)

    with tile.TileContext(nc) as tc:
        with (
            tc.tile_pool(name="consts", bufs=1) as consts,
            tc.tile_pool(name="xin", bufs=3) as xin,
            tc.tile_pool(name="qkp", bufs=2) as qkp,
            tc.tile_pool(name="vp", bufs=2) as vp,
            tc.tile_pool(name="attnp", bufs=4) as attnp,
            tc.tile_pool(name="up", bufs=2) as up,
            tc.tile_pool(name="outp", bufs=2) as outp,
            tc.tile_pool(name="yp", bufs=2) as yp,
            # PSUM: s 4 banks + cs 1 + o 1 + mm 2 = 8 banks exactly
            tc.tile_pool(name="ps_s", bufs=1, space="PSUM") as ps_s,
            tc.tile_pool(name="ps_cs", bufs=1, space="PSUM") as ps_cs,
            tc.tile_pool(name="ps_o", bufs=1, space="PSUM") as ps_o,
            tc.tile_pool(name="ps_mm", bufs=2, space="PSUM") as ps_mm,
        ):
            # ---- constants ----
            # order matters: the sync DMA queue serializes, so load what the
            # prologue projections need (weights + xT(0)) before the big eb
            # table.  eb itself is chunked so block 0's patterns arrive early.
            wqk_sb = consts.tile([128, 3, 2 * D], _BF)
            nc.sync.dma_start(out=wqk_sb, in_=wqk_d[:, :, :])
            wv_sb = consts.tile([128, 3, D], _BF)
            nc.sync.dma_start(out=wv_sb, in_=wv_d[:, :, :])
            pw_sb = consts.tile([128, 3, D], _BF)
            nc.sync.dma_start(out=pw_sb, in_=pw_d[:, :, :])
            ones_sb = consts.tile([PT, 32], _BF)
            nc.vector.memset(ones_sb, 1.0)
            eb_sb = consts.tile([PT, PAIR_PAT, H, PT], _BF)
            if qkv_bias_nonzero:
                bqk_sb = consts.tile([1, 2 * D], _BF)
                nc.sync.dma_start(out=bqk_sb, in_=bqk_d[:, :])
                bv_sb = consts.tile([1, D], _BF)
                nc.sync.dma_start(out=bv_sb, in_=bv_d[:, :])
                onetok_sb = consts.tile([1, NH], _BF)
                nc.vector.memset(onetok_sb, 1.0)

            # PSUM->SBUF copy dispatcher: 8/13 scalar, 5/13 vector
            copy_cycle = itertools.cycle(
                [0, 1, 0, 0, 1, 0, 1, 0, 0, 1, 0, 1, 0]
            )

            def evac(out, in_):
                if next(copy_cycle) == 0:
                    nc.scalar.copy(out=out, in_=in_)
                else:
                    nc.vector.tensor_copy(out=out, in_=in_)

            # ---- per-block tile state ----
            xT_sb = [None] * N_BLK
            qk_sb = [None] * N_BLK
            v_sb = [None] * N_BLK
            outN_sb = [None] * N_BLK
            yT_sb = [None] * N_BLK
            attn_of = {}   # pair index -> attn tile
            ebmul_pending = {}   # pair index -> (attn_out, attn_exp)

            def dma_x(b):
                xT_sb[b] = xin.tile([128, 3, BLK_T], _BF, tag="x", name=f"xT{b}")
                # one DMA per k-chunk: transfers spread across queues and
                # consumers (which read a single k slice) unblock earlier
                for k in range(3):
                    nc.sync.dma_start(
                        out=xT_sb[b][:, k, :],
                        in_=xT_view[:, k, b * BLK_T : (b + 1) * BLK_T],
                    )

            def qk_group(b, m, nh):
                mm_ps = ps_mm.tile([128, NH], _F32, tag="mm", name=f"mm{b}_{m}_{nh}")
                for k in range(3):
                    nc.tensor.matmul(
                        out=mm_ps,
                        lhsT=wqk_sb[:, k, 128 * m : 128 * (m + 1)],
                        rhs=xT_sb[b][:, k, nh * NH : (nh + 1) * NH],
                        start=(k == 0),
                        stop=(k == 2) if not qkv_bias_nonzero else False,
                    )
                if qkv_bias_nonzero:
                    nc.tensor.matmul(
                        out=mm_ps,
                        lhsT=bqk_sb[:, 128 * m : 128 * (m + 1)],
                        rhs=onetok_sb,
                        start=False,
                        stop=True,
                    )
                evac(qk_sb[b][:, m, nh * NH : (nh + 1) * NH], mm_ps)

            def v_group(b, p8):
                v_ps = ps_mm.tile([PT, D], _F32, tag="mm", name="v_ps")
                for k in range(3):
                    nc.tensor.matmul(
                        out=v_ps,
                        lhsT=xT_sb[b][:, k, p8 * PT : (p8 + 1) * PT],
                        rhs=wv_sb[:, k, :],
                        start=(k == 0),
                        stop=(k == 2) if not qkv_bias_nonzero else False,
                    )
                if qkv_bias_nonzero:
                    nc.tensor.matmul(
                        out=v_ps,
                        lhsT=onetok_sb[:, :PT],
                        rhs=bv_sb,
                        start=False,
                        stop=True,
                    )
                evac(v_sb[b][:, p8, :], v_ps)

            def proj_group(b, m, nh):
                y_ps = ps_mm.tile([128, NH], _F32, tag="mm", name="y_ps")
                for k in range(3):
                    nc.tensor.matmul(
                        out=y_ps,
                        lhsT=pw_sb[:, k, 128 * m : 128 * (m + 1)],
                        rhs=outN_sb[b][:, k, nh * 4 : (nh + 1) * 4, :],
                        start=(k == 0),
                        stop=(k == 2),
                    )
                evac(yT_sb[b][:, m, nh * NH : (nh + 1) * NH], y_ps)

            def scores_and_exp(i):
                """PE scores + ACT exp + DVE *EB for pair i."""
                b, p8 = i // 8, i % 8
                ts = p8 * PT
                # one 4-bank PSUM tile; bank j <- row-group j only
                # (3 used regions per bank at 512B offsets), so concurrent
                # row-groups hit different banks.
                s_ps = ps_s.tile([PT, 4, 4, 128], _F32, tag="s", name="s4")
                for r in range(3):
                    for j in range(4):
                        g = (j + r) % 3
                        nc.tensor.matmul(
                            out=s_ps[:, j, g, :PT],
                            lhsT=qk_sb[b][
                                32 * j : 32 * (j + 1), 3 + g, ts : ts + PT
                            ],
                            rhs=qk_sb[b][32 * j : 32 * (j + 1), g, ts : ts + PT],
                            start=True,
                            stop=True,
                            tile_position=(32 * j, 0),
                        )
                # one merged exp over all 12 heads (attn idx 3j+g = head 4g+j)
                attn_e = attnp.tile([PT, H, PT], _BF, tag="attne", name=f"attne{i}")
                attn_v = attn_e.rearrange("p (j g) c -> p j g c", g=3)
                nc.scalar.activation(
                    out=attn_v,
                    in_=s_ps[:, :, :3, :PT],
                    func=mybir.ActivationFunctionType.Exp,
                )
                # out-of-place *EB keeps the DVE in its fast 2x mode; the
                # multiply itself is emitted at the END of the slot (see the
                # main loop) so the DVE's in-order queue never stalls on
                # exp(i) while ready work (recip/norm of pair i-LAG) waits.
                attn_sb = attnp.tile([PT, H, PT], _BF, tag="attn", name=f"attn{i}")
                attn_of[i] = attn_sb
                ebmul_pending[i] = (attn_sb, attn_e)

            def reduce_pair(j):
                """colsum/AV on PE + 1/Z + normalize on DVE for pair j."""
                b, p8 = j // 8, j % 8
                attn_sb = attn_of.pop(j)
                # softmax denominators: 4 col-tiled matmuls, 3 heads each
                cs_ps = ps_cs.tile([128, 3, PT], _F32, tag="cs", name=f"cs{j}")
                for c in range(4):
                    nc.tensor.matmul(
                        out=cs_ps[32 * c : 32 * (c + 1), :, :],
                        lhsT=ones_sb,
                        rhs=attn_sb[:, 3 * c : 3 * (c + 1), :],
                        start=True,
                        stop=True,
                        tile_position=(0, 32 * c),
                    )
                u_sb = up.tile([128, 3, PT], _F32, tag="u", name=f"u{j}")
                nc.vector.reciprocal_approx_fast(out=u_sb, in_=cs_ps)

                # attn @ v (unnormalized), col-tiled by head
                o_ps = ps_o.tile([128, 3, PT], _F32, tag="o", name=f"o{j}")
                for r in range(4):
                    for g in range(3):
                        jj = (g + r) % 4
                        h = 4 * g + jj
                        nc.tensor.matmul(
                            out=o_ps[32 * jj : 32 * (jj + 1), g, :],
                            lhsT=v_sb[b][:, p8, 32 * h : 32 * (h + 1)],
                            rhs=attn_sb[:, 3 * jj + g, :],
                            start=True,
                            stop=True,
                            tile_position=(0, 32 * jj),
                        )
                # normalize: outN = o * (1/Z)
                nc.vector.tensor_mul(outN_sb[b][:, :, p8, :], o_ps, u_sb)

            def alloc_block(b):
                qk_sb[b] = qkp.tile([128, 6, BLK_T], _BF, tag="qk", name=f"qk{b}")
                v_sb[b] = vp.tile([PT, BLK_PAIRS, D], _BF, tag="v", name=f"v{b}")

            def alloc_out(b):
                outN_sb[b] = outp.tile([128, 3, BLK_PAIRS, PT], _BF, tag="outN", name=f"outN{b}")
                yT_sb[b] = yp.tile([128, 3, BLK_T], _BF, tag="yT", name=f"yT{b}")

            # ---- prologue: block 0 projections, first DMAs ----
            dma_x(0)
            nc.sync.dma_start(out=eb_sb[:, 0:8], in_=eb_d[:, 0:8])
            if N_BLK > 1:
                dma_x(1)
            for c in range(1, 4):
                nc.sync.dma_start(
                    out=eb_sb[:, 8 * c : 8 * (c + 1)],
                    in_=eb_d[:, 8 * c : 8 * (c + 1)],
                )
            alloc_block(0)
            # only what pairs 0-3 need; the rest of block 0's projections
            # become iteration-0 interleave units
            for m in range(6):
                qk_group(0, m, 0)
            for p8 in range(2):
                v_group(0, p8)

            # ---- pipelined main loop ----
            for B in range(N_BLK + 1):
                if B + 2 < N_BLK:
                    dma_x(B + 2)
                if B + 1 < N_BLK:
                    alloc_block(B + 1)
                if B < N_BLK:
                    alloc_out(B)

                # interleavable work units for this iteration
                units = []
                if B == 0:
                    for p8 in range(2, 4):
                        units.append(("v", 0, p8, None))
                    for m in range(6):
                        units.append(("qk", 0, m, 1))
                    for p8 in range(4, 8):
                        units.append(("v", 0, p8, None))
                if B + 1 < N_BLK:
                    for m in range(6):
                        units.append(("qk", B + 1, m, 0))
                if B >= 1:
                    for m in range(3):
                        units.append(("proj", B - 1, m, 0))
                if B + 1 < N_BLK:
                    for p8 in range(4):
                        units.append(("v", B + 1, p8, None))
                    for m in range(6):
                        units.append(("qk", B + 1, m, 1))
                if B >= 1:
                    for m in range(3):
                        units.append(("proj", B - 1, m, 1))
                if B + 1 < N_BLK:
                    for p8 in range(4, 8):
                        units.append(("v", B + 1, p8, None))

                nu = len(units)
                for s in range(8):
                    i = 8 * B + s
                    if i < N_PAIR:
                        scores_and_exp(i)
                    j = i - LAG
                    if 0 <= j < N_PAIR:
                        reduce_pair(j)
                    for u in units[(nu * s + 7) // 8 : (nu * (s + 1) + 7) // 8]:
                        kind, bb, m, nh = u
                        if kind == "qk":
                            qk_group(bb, m, nh)
                        elif kind == "v":
                            v_group(bb, m)
                        else:
                            proj_group(bb, m, nh)
                    if i in ebmul_pending:
                        attn_sb, attn_e = ebmul_pending.pop(i)
                        pr = i % PAIR_PAT
                        nc.vector.tensor_mul(attn_sb, attn_e, eb_sb[:, pr, :, :])

                if B >= 1:
                    nc.sync.dma_start(
                        out=yT_view[:, :, (B - 1) * BLK_T : B * BLK_T],
                        in_=yT_sb[B - 1],
                    )

    nc.compile()
    return nc


_NC_CACHE: dict = {}


def _get_nc(qkv_bias_nonzero: bool):
    key = qkv_bias_nonzero
    if key not in _NC_CACHE:
        _NC_CACHE[key] = _build_nc(qkv_bias_nonzero)
    return _NC_CACHE[key]


def _host_prep(x, mask, qkv_w, qkv_b, proj_w, rpb_table):
    """Build per-core input maps (all device tensors bf16)."""
    # x^T per core: [384, 12544], channel-major
    x8 = np.ascontiguousarray(x, dtype=F32).reshape(N_CORES, T_CORE, D)

    # weights: lhsT layout [ci, co] chunked as [128, 3, co]
    wqkv_t = np.ascontiguousarray(qkv_w, dtype=F32).T  # [384, 1152] = [ci, co]
    wqk = wqkv_t[:, : 2 * D].copy()
    wqk[:, :D] *= SCALE  # fold 1/sqrt(hd) into q weights
    wv = wqkv_t[:, 2 * D :]
    pw_t = np.ascontiguousarray(proj_w, dtype=F32).T  # [ci, co]

    def chunk(w):  # [384, co] -> [128, 3, co]
        return np.ascontiguousarray(
            w.reshape(3, 128, w.shape[1]).transpose(1, 0, 2)
        ).astype(BF16)

    wqk_a, wv_a, pw_a = chunk(wqk), chunk(wv), chunk(pw_t)

    # EB = exp(biasT + maskT) per pair pattern, [98, 32, 12, 98] (j, pr, h, i)
    rpi = _relative_position_index()
    bias = np.asarray(rpb_table, dtype=F32)[rpi]          # [i, j, H]
    biasT = bias.transpose(2, 1, 0)                        # [H, j, i]
    maskT = np.asarray(mask, dtype=F32).transpose(0, 2, 1)  # [w, j, i]
    mb = np.full((PAIR_PAT, H, PT, PT), -30000.0, dtype=F32)
    mb[:, :, :WN, :WN] = biasT[None] + maskT[0::2, None, :, :]
    mb[:, :, WN:, WN:] = biasT[None] + maskT[1::2, None, :, :]
    eb = np.exp(mb)
    # device head order is (j, g): idx = 3j + g holds head h = 4g + j
    perm = np.array([4 * (i % 3) + i // 3 for i in range(H)])
    eb = eb[:, perm]
    eb_a = np.ascontiguousarray(eb.transpose(2, 0, 1, 3)).astype(BF16)

    b = np.asarray(qkv_b, dtype=F32)
    bqk = b[: 2 * D].copy()
    bqk[:D] *= SCALE
    bqk_a = bqk[None, :].astype(BF16)
    bv_a = b[2 * D :][None, :].astype(BF16)

    in_maps = []
    for c in range(N_CORES):
        xT_c = np.ascontiguousarray(x8[c].T).astype(BF16)  # [384, 12544]
        in_maps.append(
            {
                "xT": xT_c,
                "wqk": wqk_a,
                "wv": wv_a,
                "pw": pw_a,
                "eb": eb_a,
                "bqk": bqk_a,
                "bv": bv_a,
            }
        )
    return in_maps


def kernel(x, mask, qkv_w, qkv_b, proj_w, rpb_table, _want_trace=False):
    qkv_bias_nonzero = bool(np.any(np.asarray(qkv_b) != 0))
    nc = _get_nc(qkv_bias_nonzero)
    in_maps = _host_prep(x, mask, qkv_w, qkv_b, proj_w, rpb_table)
    res = run_bass_kernel_spmd(
        nc, in_maps, core_ids=list(range(N_CORES)), trace=_want_trace
    )
    yT = np.stack([res.results[c]["yT"].astype(F32) for c in range(N_CORES)])
    y = yT.transpose(0, 2, 1).reshape(B_, WN, D)
    if _want_trace:
        kernel._last_result = res
    return y
